# revision 1
# baseline (speedup 1.0000x reference)
"""Trainium2 Bass kernel for a dense transformer decoder block.

reference: x:(2,2048,1024) f32; LN1 -> causal MHA (16 heads, d=64) -> +res;
LN2 -> MLP (d_ff=4096, exact gelu) -> +res.

Sharding: token-parallel over the 4096 (batch*seq) tokens, 512 tokens/core.
Cores 0-3 own batch 0, cores 4-7 batch 1 (chunk c = core % 4).  Each core
projects q/k/v for its own tokens; K and V are exchanged with AllGather
collectives over replica groups [[0..3],[4..7]] so every core holds its
batch's full 2048-position K/V.  Causality is enforced by a per-core mask
threshold applied with the DVE TENSOR_MASK op after exp (select -> exact
zeros, no inf*0).  Softmax runs without max-subtraction (logits are small);
denominators come from a ones-column appended to V (PV matmul M=65, row 64
accumulates sum(exp)).

LayerNorm weights are folded into w_qkv / w_fc1 on the host.  Matmul inputs
are bf16 (fp32 PSUM accumulation); residual stream and softmax statistics
stay fp32.
"""

import numpy as np

B, S, H = 2, 2048, 1024
NCORES = 8
TOK = 512            # tokens per core
HEADS, D = 16, 64
DFF = 4096
EPS = 1e-5

_PROG = None         # cached compiled program


def _build_program(debug=False):
    import concourse.mybir as mybir
    import concourse.tile as tile
    from concourse import bacc
    from concourse.masks import make_identity

    dt = mybir.dt
    f32, bf16 = dt.float32, dt.bfloat16
    AF = mybir.ActivationFunctionType
    ALU = mybir.AluOpType

    nc = bacc.Bacc("TRN2", target_bir_lowering=False, debug=False,
                   num_devices=NCORES)

    # ---- kernel I/O (weights pre-rearranged on host for contiguous DMA) ----
    xc_d = nc.dram_tensor("xc", [TOK, H], f32, kind="ExternalInput")
    # wqk[p, ot, hc, o]  = (ln1w*wqkv)[128*hc+p, 128*ot+o]   (q,k cols 0..2047)
    wqk_d = nc.dram_tensor("wqk", [128, 16, 8, 128], bf16, kind="ExternalInput")
    # wv[p, vn, hc, o]   = (ln1w*wqkv)[128*hc+p, 2048+512*vn+o]
    wv_d = nc.dram_tensor("wv", [128, 2, 8, TOK], bf16, kind="ExternalInput")
    # wout[d, no, h, o]  = w_out[64*h+d, 512*no+o]
    wout_d = nc.dram_tensor("wout", [64, 2, HEADS, TOK], bf16,
                            kind="ExternalInput")
    # wfc1[p, f, hc, o]  = (ln2w*w_fc1)[128*hc+p, 128*f+o]
    wfc1_d = nc.dram_tensor("wfc1", [128, 32, 8, 128], bf16,
                            kind="ExternalInput")
    wfc2_d = nc.dram_tensor("wfc2", [DFF, H], bf16, kind="ExternalInput")
    # mk[p, 16*j + q/512...] causal mask: 1 where (128j+p) <= (512c+q)
    mk_d = nc.dram_tensor("mk", [128, 16, TOK], bf16, kind="ExternalInput")
    out_d = nc.dram_tensor("out", [TOK, H], f32, kind="ExternalOutput")
    if debug:
        dat_d = nc.dram_tensor("dbg_at", [64, HEADS * TOK], bf16,
                               kind="ExternalOutput")
        dden_d = nc.dram_tensor("dbg_den", [HEADS, TOK], f32,
                                kind="ExternalOutput")
        dex_d = nc.dram_tensor("dbg_ex", [128, 16 * 1024], bf16,
                               kind="ExternalOutput")

    GROUPS = [[0, 1, 2, 3], [4, 5, 6, 7]]

    with tile.TileContext(nc) as tc:
        with (
            tc.tile_pool(name="persist", bufs=1) as pp,
            tc.tile_pool(name="dram", bufs=1, space="DRAM") as dram,
        ):
            # ---------- DRAM bounce buffers for collectives ----------
            kb_d = dram.tile([H, TOK], bf16, tag="kb_d")
            kg_d = dram.tile([4 * H, TOK], bf16, tag="kg_d")
            vb_d = dram.tile([TOK, 65 * HEADS], bf16, tag="vb_d")
            vg_d = dram.tile([4 * TOK, 65 * HEADS], bf16, tag="vg_d")

            # ---------- persistent SBUF tensors ----------
            x_sb = pp.tile([128, 4, H], f32, tag="x_sb")      # x[128*tt+p, h]
            qt = pp.tile([128, 8, TOK], bf16, tag="qt")       # qT
            kb = pp.tile([128, 8, TOK], bf16, tag="kb")       # kT stage
            kt = pp.tile([128, 4, 8, TOK], bf16, tag="big32")  # gathered kT
            v_sb = pp.tile([128, 16, 65 * HEADS], bf16, tag="v_sb")
            at = pp.tile([64, HEADS, TOK], bf16, tag="at")    # attn outT
            mk_sb = pp.tile([128, 16, TOK], bf16, tag="mk")
            ones64 = pp.tile([128, 64], bf16, tag="ones64")
            ident = pp.tile([128, 128], bf16, tag="ident")
            eps_sb = pp.tile([128, 1], f32, tag="eps_sb")
            nb50 = pp.tile([128, 1], f32, tag="nb50")

            # ---------- constants ----------
            nc.sync.dma_start(mk_sb[:], mk_d.ap())
            nc.gpsimd.memset(ones64[:], 1.0)
            nc.gpsimd.memset(eps_sb[:], EPS)
            nc.gpsimd.memset(nb50[:], -25.0)
            make_identity(nc, ident[:])

            # ---------- load x ----------
            nc.sync.dma_start(
                x_sb[:], xc_d.ap().rearrange("(tt p) h -> p tt h", p=128))

            # ---------- LayerNorm + transpose helper ----------
            def layernorm_t(xlt_dst, ln_pool, tp_ps_pool):
                for tt in range(4):
                    xrow = x_sb[:, tt, :]
                    st = ln_pool.tile([128, 2, 6], f32, tag="ln_st")
                    nc.vector.bn_stats(st[:, 0, :], xrow[:, 0:512])
                    nc.vector.bn_stats(st[:, 1, :], xrow[:, 512:1024])
                    agg = ln_pool.tile([128, 2], f32, tag="ln_agg")
                    nc.vector.bn_aggr(agg[:], st[:])
                    rstd = ln_pool.tile([128, 1], f32, tag="ln_rstd")
                    nmr = ln_pool.tile([128, 1], f32, tag="ln_nmr")
                    # rstd = exp(-0.5 * ln(var + eps))
                    nc.scalar.activation(rstd[:], agg[:, 1:2], AF.Ln,
                                         bias=eps_sb[:])
                    nc.scalar.activation(rstd[:], rstd[:], AF.Exp, scale=-0.5)
                    nc.vector.tensor_tensor(nmr[:], agg[:, 0:1], rstd[:],
                                            ALU.mult)
                    nc.vector.tensor_scalar_mul(nmr[:], nmr[:], -1.0)
                    xln = ln_pool.tile([128, H], bf16, tag="ln_out")
                    nc.scalar.activation(xln[:], xrow, AF.Identity,
                                         bias=nmr[:], scale=rstd[:])
                    for hc in range(8):
                        tp = tp_ps_pool.tile([128, 128], bf16, tag="tp")
                        nc.tensor.transpose(
                            tp[:], xln[:, 128 * hc:128 * (hc + 1)], ident[:])
                        nc.vector.tensor_copy(
                            xlt_dst[:, hc, 128 * tt:128 * (tt + 1)], tp[:])

            with (
                tc.tile_pool(name="ln1", bufs=2) as ln_pool,
                tc.tile_pool(name="tp_ps", bufs=2, space="PSUM") as tp_ps,
            ):
                xlt = pp.tile([128, 8, TOK], bf16, tag="xlt")
                layernorm_t(xlt, ln_pool, tp_ps)

            # ---------- q/k/v projections ----------
            with (
                tc.tile_pool(name="wqk_p", bufs=3) as wqk_p,
                tc.tile_pool(name="wv_p", bufs=2) as wv_p,
                tc.tile_pool(name="mm_ps", bufs=4, space="PSUM") as mm_ps,
                tc.tile_pool(name="vaux", bufs=2) as vaux,
            ):
                # qT / kT: out[qf, t] = sum_hin wqkv[hin, qf] * xlnT[hin, t]
                for ot in range(16):
                    wq = wqk_p.tile([128, 8, 128], bf16, tag="wqk")
                    nc.sync.dma_start(wq[:], wqk_d.ap()[:, ot, :, :])
                    ps = mm_ps.tile([128, TOK], f32, tag="mm")
                    for hc in range(8):
                        nc.tensor.matmul(ps[:], wq[:, hc, :], xlt[:, hc, :],
                                         start=(hc == 0), stop=(hc == 7))
                    dst = qt[:, ot, :] if ot < 8 else kb[:, ot - 8, :]
                    nc.vector.tensor_copy(dst, ps[:])
                nc.sync.dma_start(
                    kb_d[:].rearrange("(ot p) t -> p ot t", p=128), kb[:])

                # V token-major with ones column per head
                wvt = []
                for vn in range(2):
                    wv = wv_p.tile([128, 8, TOK], bf16, tag="wv")
                    nc.sync.dma_start(wv[:], wv_d.ap()[:, vn, :, :])
                    wvt.append(wv)
                vb_view = vb_d[:].rearrange("(tt p) f -> p tt f", p=128)
                for tt in range(4):
                    va = vaux.tile([128, HEADS, 65], bf16, tag="vaug")
                    nc.gpsimd.memset(va[:, :, 64:65], 1.0)
                    for vn in range(2):
                        ps = mm_ps.tile([128, TOK], f32, tag="mm")
                        for hc in range(8):
                            nc.tensor.matmul(
                                ps[:], xlt[:, hc, 128 * tt:128 * (tt + 1)],
                                wvt[vn][:, hc, :], start=(hc == 0),
                                stop=(hc == 7))
                        nc.vector.tensor_copy(
                            va[:, 8 * vn:8 * (vn + 1), 0:64],
                            ps[:].rearrange("p (h e) -> p h e", e=64))
                    nc.sync.dma_start(
                        vb_view[:, tt, :],
                        va[:].rearrange("p h e -> p (h e)"))

            # ---------- collectives ----------
            nc.gpsimd.collective_compute(
                "AllGather", mybir.AluOpType.bypass, replica_groups=GROUPS,
                ins=[kb_d[:].opt()], outs=[kg_d[:].opt()])
            nc.gpsimd.collective_compute(
                "AllGather", mybir.AluOpType.bypass, replica_groups=GROUPS,
                ins=[vb_d[:].opt()], outs=[vg_d[:].opt()])

            nc.sync.dma_start(
                kt[:], kg_d[:].rearrange("(r hh hp d) t -> (hp d) r hh t",
                                         r=4, hh=8, hp=2, d=64))
            nc.sync.dma_start(
                v_sb[:], vg_d[:].rearrange("(r tt p) f -> p (r tt) f",
                                           r=4, tt=4, p=128))

            # ---------- attention + out projection ----------
            with (
                tc.tile_pool(name="qk_ps", bufs=2, space="PSUM") as qk_ps,
                tc.tile_pool(name="pv_ps", bufs=2, space="PSUM") as pv_ps,
                tc.tile_pool(name="br_ps", bufs=1, space="PSUM") as br_ps,
                tc.tile_pool(name="exp_p", bufs=4) as exp_p,
                tc.tile_pool(name="den_p", bufs=2) as den_p,
                tc.tile_pool(name="rb_p", bufs=2) as rb_p,
                tc.tile_pool(name="wo_p", bufs=1) as wo_p,
                tc.tile_pool(name="mm2_ps", bufs=1, space="PSUM") as mm2_ps,
            ):
                for hh in range(8):          # head pairs (2hh, 2hh+1)
                    psO = [pv_ps.tile([65, TOK], f32, tag="pv",
                                      name=f"psO_{hh}_{hp}")
                           for hp in range(2)]
                    for j in range(16):      # kv chunks of 128
                        r, cc = j // 4, j % 4
                        ps = qk_ps.tile([128, 1024], f32, tag="qk")
                        for hp in range(2):  # row-tiled head pair
                            nc.tensor.matmul(
                                ps[:, TOK * hp:TOK * (hp + 1)],
                                kt[64 * hp:64 * (hp + 1), r, hh,
                                   128 * cc:128 * (cc + 1)],
                                qt[64 * hp:64 * (hp + 1), hh, :],
                                start=True, stop=True)
                        ex = exp_p.tile([128, 1024], bf16, tag="exp")
                        # -25 bias: keeps garbage (masked-pos) exps finite
                        # (mask-mul can't make inf*0) while denominators stay
                        # big enough for the ACT Ln table range; cancels in
                        # the softmax.
                        nc.scalar.activation(ex[:], ps[:], AF.Exp,
                                             scale=0.125, bias=nb50[:])
                        nc.vector.tensor_tensor(ex[:, 0:TOK], ex[:, 0:TOK],
                                                mk_sb[:, j, :], ALU.mult)
                        nc.gpsimd.tensor_tensor(ex[:, TOK:1024],
                                                ex[:, TOK:1024],
                                                mk_sb[:, j, :], ALU.mult)
                        if debug and hh == 0:
                            nc.sync.dma_start(
                                dex_d.ap()
                                .rearrange("p (j k) -> p j k", j=16)[:, j, :],
                                ex[:])
                        for hp in range(2):
                            h = 2 * hh + hp
                            nc.tensor.matmul(
                                psO[hp][:], v_sb[:, j, 65 * h:65 * (h + 1)],
                                ex[:, TOK * hp:TOK * (hp + 1)],
                                start=(j == 0), stop=(j == 15))
                    if debug:
                        for hp in range(2):
                            dcp = den_p.tile([128, TOK], f32, tag="dbgden",
                                             name=f"dbgden_{hh}_{hp}")
                            nc.vector.tensor_copy(dcp[64:65, :],
                                                  psO[hp][64:65, :])
                            nc.sync.dma_start(
                                dden_d.ap()[2 * hh + hp:2 * hh + hp + 1, :],
                                dcp[64:65, :])
                    for hp in range(2):      # normalize -> at
                        h = 2 * hh + hp
                        # 1/den = exp(-ln(den)); same ACT table set as Exp
                        den = den_p.tile([128, TOK], f32, tag="den")
                        nc.scalar.activation(den[64:65, :],
                                             psO[hp][64:65, :], AF.Ln)
                        denb = den_p.tile([128, TOK], bf16, tag="denb")
                        nc.scalar.activation(denb[64:65, :], den[64:65, :],
                                             AF.Exp, scale=-1.0)
                        psb = br_ps.tile([64, TOK], f32, tag="br")
                        nc.tensor.matmul(psb[:], ones64[64:65, 0:64],
                                         denb[64:65, :], start=True, stop=True)
                        rb = rb_p.tile([64, TOK], bf16, tag="rb")
                        nc.vector.tensor_copy(rb[:], psb[:])
                        nc.vector.tensor_tensor(at[:, h, :], psO[hp][0:64, :],
                                                rb[:], ALU.mult)

                if debug:
                    nc.sync.dma_start(
                        dat_d.ap(),
                        at[:].rearrange("d h t -> d (h t)"))
                # out projection + residual (x2 written in place over x_sb)
                for no in range(2):
                    wo = wo_p.tile([64, HEADS, TOK], bf16, tag="wo")
                    nc.sync.dma_start(wo[:], wout_d.ap()[:, no, :, :])
                    for tt in range(4):
                        ps = mm2_ps.tile([128, TOK], f32, tag="mm2")
                        for h in range(HEADS):
                            nc.tensor.matmul(
                                ps[:], at[:, h, 128 * tt:128 * (tt + 1)],
                                wo[:, h, :], start=(h == 0),
                                stop=(h == HEADS - 1))
                        nc.vector.tensor_tensor(
                            x_sb[:, tt, TOK * no:TOK * (no + 1)], ps[:],
                            x_sb[:, tt, TOK * no:TOK * (no + 1)], ALU.add)

            # ---------- LN2 + transpose ----------
            with (
                tc.tile_pool(name="ln2", bufs=2) as ln2_pool,
                tc.tile_pool(name="tp2_ps", bufs=2, space="PSUM") as tp2_ps,
            ):
                xlt2 = pp.tile([128, 8, TOK], bf16, tag="xlt")
                layernorm_t(xlt2, ln2_pool, tp2_ps)

            # ---------- MLP ----------
            h1g = pp.tile([128, 32, TOK], bf16, tag="big32")
            with (
                tc.tile_pool(name="wfc1_p", bufs=3) as wfc1_p,
                tc.tile_pool(name="fc1_ps", bufs=2, space="PSUM") as fc1_ps,
            ):
                for f in range(32):
                    wt = wfc1_p.tile([128, 8, 128], bf16, tag="wfc1")
                    nc.sync.dma_start(wt[:], wfc1_d.ap()[:, f, :, :])
                    ps = fc1_ps.tile([128, TOK], f32, tag="fc1")
                    for hc in range(8):
                        nc.tensor.matmul(ps[:], wt[:, hc, :], xlt2[:, hc, :],
                                         start=(hc == 0), stop=(hc == 7))
                    nc.scalar.activation(h1g[:, f, :], ps[:], AF.Gelu)

            with (
                tc.tile_pool(name="wfc2_p", bufs=3) as wfc2_p,
                tc.tile_pool(name="fc2_ps", bufs=4, space="PSUM") as fc2_ps,
                tc.tile_pool(name="o_p", bufs=2) as o_p,
            ):
                out_view = out_d.ap().rearrange("(tt p) h -> p tt h", p=128)
                for no in range(2):
                    pss = [fc2_ps.tile([128, TOK], f32, tag="fc2",
                                       name=f"fc2ps_{no}_{tt}")
                           for tt in range(4)]
                    for f in range(32):
                        wt = wfc2_p.tile([128, TOK], bf16, tag="wfc2")
                        nc.sync.dma_start(
                            wt[:], wfc2_d.ap()[128 * f:128 * (f + 1),
                                               TOK * no:TOK * (no + 1)])
                        for tt in range(4):
                            nc.tensor.matmul(
                                pss[tt], h1g[:, f, 128 * tt:128 * (tt + 1)],
                                wt[:], start=(f == 0), stop=(f == 31))
                    for tt in range(4):
                        o = o_p.tile([128, TOK], f32, tag="o")
                        nc.vector.tensor_tensor(
                            o[:], pss[tt],
                            x_sb[:, tt, TOK * no:TOK * (no + 1)], ALU.add)
                        nc.sync.dma_start(
                            out_view[:, tt, TOK * no:TOK * (no + 1)], o[:])

    nc.compile()
    return nc


def _host_prepare(x, ln1_w, ln2_w, w_qkv, w_out, w_fc1, w_fc2):
    """Fold LN weights into the following matmuls, cast to bf16, and
    rearrange weights into the layouts the kernel DMAs expect."""
    import ml_dtypes
    bf16 = ml_dtypes.bfloat16

    x = np.asarray(x, np.float32)
    wqkv_f = (np.asarray(ln1_w, np.float32)[:, None]
              * np.asarray(w_qkv, np.float32))
    wfc1_f = (np.asarray(ln2_w, np.float32)[:, None]
              * np.asarray(w_fc1, np.float32))

    # wqk[p, ot, hc, o] = wqkv_f[128*hc+p, 128*ot+o]
    wqk = np.ascontiguousarray(
        wqkv_f[:, :2048].reshape(8, 128, 16, 128).transpose(1, 2, 0, 3)
    ).astype(bf16)
    # wv[p, vn, hc, o] = wqkv_f[128*hc+p, 2048+512*vn+o]
    wv = np.ascontiguousarray(
        wqkv_f[:, 2048:].reshape(8, 128, 2, TOK).transpose(1, 2, 0, 3)
    ).astype(bf16)
    # wout[d, no, h, o] = w_out[64*h+d, 512*no+o]
    wout = np.ascontiguousarray(
        np.asarray(w_out, np.float32).reshape(HEADS, 64, 2, TOK)
        .transpose(1, 2, 0, 3)
    ).astype(bf16)
    # wfc1[p, f, hc, o] = wfc1_f[128*hc+p, 128*f+o]
    wfc1 = np.ascontiguousarray(
        wfc1_f.reshape(8, 128, 32, 128).transpose(1, 2, 0, 3)
    ).astype(bf16)
    wfc2 = np.asarray(w_fc2, np.float32).astype(bf16)

    xf = x.reshape(B * S // TOK, TOK, H)   # 8 chunks of 512 tokens

    # causal mask per chunk c: mk[p, j, q] = (128j + p) <= (512c + q)
    p_idx = np.arange(128)[:, None, None]
    j_idx = np.arange(16)[None, :, None]
    q_idx = np.arange(TOK)[None, None, :]
    masks = [
        ((128 * j_idx + p_idx) <= (512 * c + q_idx)).astype(bf16)
        for c in range(4)
    ]

    in_maps = []
    for i in range(NCORES):
        in_maps.append({
            "xc": np.ascontiguousarray(xf[i]),
            "wqk": wqk, "wv": wv, "wout": wout,
            "wfc1": wfc1, "wfc2": wfc2,
            "mk": masks[i % 4],
        })
    return in_maps


def kernel(x, ln1_w, ln2_w, w_qkv, w_out, w_fc1, w_fc2):
    global _PROG
    from concourse.bass_utils import run_bass_kernel_spmd

    if _PROG is None:
        _PROG = _build_program()
    nc = _PROG

    in_maps = _host_prepare(x, ln1_w, ln2_w, w_qkv, w_out, w_fc1, w_fc2)
    res = run_bass_kernel_spmd(nc, in_maps, core_ids=list(range(NCORES)))
    out = np.concatenate([res.results[i]["out"][None] for i in range(NCORES)],
                         axis=0)
    return np.ascontiguousarray(out.reshape(B, S, H).astype(np.float32))



# revision 17
# speedup vs baseline: 1.4309x; 1.4309x over previous
"""Trainium2 Bass kernel for a dense transformer decoder block.

reference: x:(2,2048,1024) f32; LN1 -> causal MHA (16 heads, d=64) -> +res;
LN2 -> MLP (d_ff=4096, exact gelu) -> +res.

Sharding: stride-4 interleaved token-parallel.  Core i (batch b=i//4, rank
k=i%4) owns tokens x[b, k::4] (512 tokens).  With this assignment, local
query block t (128 queries, global positions 4*(128t+qq)+k) attends exactly
the gathered key blocks (r, jj<=t) -- the causal skip pattern is identical
on every core (40 of 64 key-block visits), so one uniform SPMD program is
load-balanced AND skips ~38% of attention work.

K/V are exchanged in fp8e4 wire format with FOUR per-head-group AllGathers
(heads 4g..4g+3) fired as soon as each group's K/V projections finish, so
the collectives pipeline under Q projection and attention on earlier
groups.  Masking is applied only to the diagonal key block (first 128
query columns of each ex tile); softmax denominators use the ones-column-
in-V trick with DVE reciprocal (no ACT table thrash).  Matmul compute is
bf16 (fp8 lhsT for attention K/V), fp32 PSUM.
"""

import numpy as np

B, S, H = 2, 2048, 1024
NCORES = 8
TOK = 512            # tokens per core
HEADS, D = 16, 64
DFF = 4096
EPS = 1e-5

KVK = 128 * 2 * 512           # K wire region [p, o, t]
KVV = 4 * 128 * 260           # V wire region [tt, p, c] (65 cols/head incl ones)
KVE = KVK + KVV

_PROG = None         # cached compiled program


def _build_program():
    import concourse.mybir as mybir
    import concourse.tile as tile
    from concourse import bacc
    from concourse.masks import make_identity

    dt = mybir.dt
    f32, bf16, f8 = dt.float32, dt.bfloat16, dt.float8e4
    AF = mybir.ActivationFunctionType
    ALU = mybir.AluOpType

    nc = bacc.Bacc("TRN2", target_bir_lowering=False, debug=False,
                   num_devices=NCORES)

    # ---- kernel I/O (weights pre-rearranged on host for contiguous DMA) ----
    xc_d = nc.dram_tensor("xc", [TOK, H], f32, kind="ExternalInput")
    # wqk[p, ot, hc, o] = (ln1w*wqkv)[128*hc+p, 128*ot+o]; ot 0-7 q, 8-15 k
    wqk_d = nc.dram_tensor("wqk", [128, 16, 8, 128], bf16, kind="ExternalInput")
    # wv[p, hc, c] = (ln1w*wqkv)[128*hc+p, 2048+c]
    wv_d = nc.dram_tensor("wv", [128, 8, 1024], bf16, kind="ExternalInput")
    # wout[64*hp+d, hh, no, o] = w_out[64*(2*hh+hp)+d, 512*no+o]
    wout_d = nc.dram_tensor("wout", [128, 8, 2, 512], bf16,
                            kind="ExternalInput")
    # wfc1[p, f, hc, o] = (ln2w*w_fc1)[128*hc+p, 128*f+o]
    wfc1_d = nc.dram_tensor("wfc1", [128, 32, 8, 128], bf16,
                            kind="ExternalInput")
    # wfc2[p, f, c] = w_fc2[128*f+p, c]
    wfc2_d = nc.dram_tensor("wfc2", [128, 32, 1024], bf16,
                            kind="ExternalInput")
    # mk[p, r, hp, qq] = 1 if 4p + r <= 4qq + k else 0   (diagonal block mask)
    mk_d = nc.dram_tensor("mk", [128, 4, 2, 128], bf16, kind="ExternalInput")
    out_d = nc.dram_tensor("out", [TOK, H], f32, kind="ExternalOutput")

    GROUPS = [[0, 1, 2, 3], [4, 5, 6, 7]]

    # per-head-group K/V exchange buffers (fp8 wire)
    kvin = [nc.dram_tensor(f"kvin{g}", [KVE], f8, kind="Internal")
            for g in range(4)]
    kvout = [nc.dram_tensor(f"kvout{g}", [4 * KVE], f8, kind="Internal")
             for g in range(4)]

    def kvin_k(g):
        return (kvin[g].ap()[0:KVK]
                .rearrange("(p o t) -> p o t", p=128, o=2, t=512))

    def kvin_v(g):
        return (kvin[g].ap()[KVK:KVE]
                .rearrange("(tt p c) -> p tt c", tt=4, p=128, c=260))

    def kvout_k(g):
        return (kvout[g].ap().rearrange("(r x) -> r x", r=4)[:, 0:KVK]
                .rearrange("r (p o t) -> p r o t", p=128, o=2, t=512))

    def kvout_v(g):
        return (kvout[g].ap().rearrange("(r x) -> r x", r=4)[:, KVK:KVE]
                .rearrange("r (tt p c) -> p r tt c", tt=4, p=128, c=260))

    with tile.TileContext(nc) as tc:
        with tc.tile_pool(name="persist", bufs=1) as pp:
            # ---------- persistent SBUF ----------
            x_sb = pp.tile([128, 4, H], f32, tag="x_sb")      # x[128*tt+p, h]
            xlt = pp.tile([128, 8, TOK], bf16, tag="xlt")     # ln(x)^T
            qt = pp.tile([128, 8, TOK], bf16, tag="qt")       # q^T
            at2 = pp.tile([128, 8, TOK], bf16, tag="at2")     # attn out, 2-head packed
            h1g = pp.tile([128, 32, TOK], bf16, tag="h1g")    # gelu(fc1)
            mk_sb = pp.tile([128, 4, 2, 128], bf16, tag="mk")
            wv_sb = pp.tile([128, 8, 1024], bf16, tag="wv_sb")
            wout_sb = pp.tile([128, 8, 2, TOK], bf16, tag="wout_sb")
            ones64 = pp.tile([128, 64], bf16, tag="ones64")
            ident = pp.tile([128, 128], bf16, tag="ident")
            eps_sb = pp.tile([128, 1], f32, tag="eps_sb")
            nb25 = pp.tile([128, 1], f32, tag="nb25")

            # ---------- constants / initial DMAs ----------
            nc.sync.dma_start(mk_sb[:], mk_d.ap())
            nc.sync.dma_start(
                x_sb[:], xc_d.ap().rearrange("(tt p) h -> p tt h", p=128))
            nc.sync.dma_start(wv_sb[:], wv_d.ap())
            nc.gpsimd.memset(ones64[:], 1.0)
            nc.gpsimd.memset(eps_sb[:], EPS)
            nc.gpsimd.memset(nb25[:], -25.0)
            make_identity(nc, ident[:])

            # ---------- LayerNorm + transpose helper ----------
            def layernorm_t(xlt_dst, ln_pool, tp_ps_pool):
                for tt in range(4):
                    xrow = x_sb[:, tt, :]
                    st = ln_pool.tile([128, 2, 6], f32, tag="ln_st")
                    nc.vector.bn_stats(st[:, 0, :], xrow[:, 0:512])
                    nc.vector.bn_stats(st[:, 1, :], xrow[:, 512:1024])
                    agg = ln_pool.tile([128, 2], f32, tag="ln_agg")
                    nc.vector.bn_aggr(agg[:], st[:])
                    rstd = ln_pool.tile([128, 1], f32, tag="ln_rstd")
                    nmr = ln_pool.tile([128, 1], f32, tag="ln_nmr")
                    # rstd = exp(-0.5 * ln(var + eps))
                    nc.scalar.activation(rstd[:], agg[:, 1:2], AF.Ln,
                                         bias=eps_sb[:])
                    nc.scalar.activation(rstd[:], rstd[:], AF.Exp, scale=-0.5)
                    nc.vector.tensor_tensor(nmr[:], agg[:, 0:1], rstd[:],
                                            ALU.mult)
                    nc.vector.tensor_scalar_mul(nmr[:], nmr[:], -1.0)
                    xln = ln_pool.tile([128, H], bf16, tag="ln_out")
                    nc.scalar.activation(xln[:], xrow, AF.Identity,
                                         bias=nmr[:], scale=rstd[:])
                    for hc in range(8):
                        tp = tp_ps_pool.tile([128, 128], bf16, tag="tp")
                        nc.tensor.transpose(
                            tp[:], xln[:, 128 * hc:128 * (hc + 1)], ident[:])
                        nc.vector.tensor_copy(
                            xlt_dst[:, hc, 128 * tt:128 * (tt + 1)], tp[:])

            with (
                tc.tile_pool(name="ln1", bufs=2) as ln_pool,
                tc.tile_pool(name="tp_ps", bufs=2, space="PSUM") as tp_ps,
            ):
                layernorm_t(xlt, ln_pool, tp_ps)

            # ---------- K + V projections per head group; fire AllGathers ----
            with (
                tc.tile_pool(name="wqk_p", bufs=3) as wqk_p,
                tc.tile_pool(name="mm_ps", bufs=3, space="PSUM") as mm_ps,
                tc.tile_pool(name="kstage", bufs=2) as kstage,
                tc.tile_pool(name="vstage", bufs=2) as vstage,
            ):
                for g in range(4):
                    kb = kstage.tile([128, 2, TOK], f8, tag="kb")
                    for oo in range(2):
                        ot = 8 + 2 * g + oo
                        wq = wqk_p.tile([128, 8, 128], bf16, tag="wqk")
                        nc.sync.dma_start(wq[:], wqk_d.ap()[:, ot, :, :])
                        ps = mm_ps.tile([128, TOK], f32, tag="mm")
                        for hc in range(8):
                            nc.tensor.matmul(ps[:], wq[:, hc, :], xlt[:, hc, :],
                                             start=(hc == 0), stop=(hc == 7))
                        nc.vector.tensor_copy(kb[:, oo, :], ps[:])
                    nc.sync.dma_start(kvin_k(g), kb[:])

                    va = vstage.tile([128, 4, 4, 65], f8, tag="va")
                    nc.gpsimd.memset(va[:, :, :, 64:65], 1.0)
                    for tt in range(4):
                        ps = mm_ps.tile([128, 256], f32, tag="mmv")
                        for hc in range(8):
                            nc.tensor.matmul(
                                ps[:], xlt[:, hc, 128 * tt:128 * (tt + 1)],
                                wv_sb[:, hc, 256 * g:256 * (g + 1)],
                                start=(hc == 0), stop=(hc == 7))
                        nc.vector.tensor_copy(
                            va[:, tt, :, 0:64],
                            ps[:].rearrange("p (h e) -> p h e", e=64))
                    nc.sync.dma_start(
                        kvin_v(g), va[:].rearrange("p tt h e -> p tt (h e)"))

                    nc.gpsimd.collective_compute(
                        "AllGather", mybir.AluOpType.bypass,
                        replica_groups=GROUPS,
                        ins=[kvin[g].ap()], outs=[kvout[g].ap()])

                # ---------- Q projection (overlaps first AllGathers) ----------
                for ot in range(8):
                    wq = wqk_p.tile([128, 8, 128], bf16, tag="wqk")
                    nc.sync.dma_start(wq[:], wqk_d.ap()[:, ot, :, :])
                    ps = mm_ps.tile([128, TOK], f32, tag="mm")
                    for hc in range(8):
                        nc.tensor.matmul(ps[:], wq[:, hc, :], xlt[:, hc, :],
                                         start=(hc == 0), stop=(hc == 7))
                    nc.vector.tensor_copy(qt[:, ot, :], ps[:])

            # prefetch out-proj weights while collectives run
            nc.sync.dma_start(wout_sb[:], wout_d.ap())

            # ---------- attention ----------
            with (
                tc.tile_pool(name="kt_p", bufs=2) as kt_p,
                tc.tile_pool(name="vg_p", bufs=2) as vg_p,
                tc.tile_pool(name="qk_ps", bufs=2, space="PSUM") as qk_ps,
                tc.tile_pool(name="pv_ps", bufs=1, space="PSUM") as pv_ps,
                tc.tile_pool(name="br_ps", bufs=1, space="PSUM") as br_ps,
                tc.tile_pool(name="exp_p", bufs=3) as exp_p,
                tc.tile_pool(name="den_p", bufs=2) as den_p,
                tc.tile_pool(name="rb_p", bufs=2) as rb_p,
                tc.tile_pool(name="tmp_p", bufs=2) as tmp_p,
            ):
                for g in range(4):
                    kt = kt_p.tile([128, 4, 2, TOK], f8, tag="kt")
                    nc.sync.dma_start(kt[:], kvout_k(g))
                    vg = vg_p.tile([128, 4, 4, 260], f8, tag="vg")
                    for r in range(4):
                        nc.sync.dma_start(vg[:, r, :, :],
                                          kvout_v(g)[:, r, :, :])

                    for o in range(2):
                        hh = 2 * g + o
                        psO = pv_ps.tile([65, 2, TOK], f32, tag="pv")
                        pend = None      # 1-deep software pipeline for PV
                        for j in range(16):
                            r, jj = j // 4, j % 4
                            N = 512 - 128 * jj
                            ps = qk_ps.tile([128, 2, TOK], f32, tag="qk")
                            for hp in range(2):
                                nc.tensor.matmul(
                                    ps[:, hp, 0:N],
                                    kt[64 * hp:64 * (hp + 1), r, o,
                                       128 * jj:128 * (jj + 1)],
                                    qt[64 * hp:64 * (hp + 1), hh,
                                       128 * jj:512],
                                    start=True, stop=True)
                            ex = exp_p.tile([128, 2, TOK], bf16, tag="exp")
                            # -25 bias keeps masked-position garbage exps tiny
                            # relative to nothing -- it cancels in softmax;
                            # mask-mult after exp gives exact zeros.
                            nc.scalar.activation(ex[:, :, 0:N], ps[:, :, 0:N],
                                                 AF.Exp, scale=0.125,
                                                 bias=nb25[:])
                            # only the diagonal sub-block (first 128 query
                            # cols of this tile) straddles causality
                            nc.vector.tensor_tensor(
                                ex[:, :, 0:128], ex[:, :, 0:128],
                                mk_sb[:, r, :, :], ALU.mult)
                            if pend is not None:
                                pend()
                            def make_pv(r=r, jj=jj, N=N, ex=ex, j=j):
                                def pv():
                                    for hp in range(2):
                                        hl = 2 * o + hp
                                        lhsT = vg[:, r, jj,
                                                  65 * hl:65 * (hl + 1)]
                                        if r == 3:
                                            nc.tensor.matmul(
                                                psO[:, hp,
                                                    128 * jj:128 * (jj + 1)],
                                                lhsT, ex[:, hp, 0:128],
                                                start=(j == 0), stop=True)
                                            if jj < 3:
                                                nc.tensor.matmul(
                                                    psO[:, hp,
                                                        128 * (jj + 1):512],
                                                    lhsT, ex[:, hp, 128:N],
                                                    start=(j == 0), stop=False)
                                        else:
                                            nc.tensor.matmul(
                                                psO[:, hp, 128 * jj:512],
                                                lhsT, ex[:, hp, 0:N],
                                                start=(j == 0), stop=False)
                                return pv
                            pend = make_pv()
                        pend()

                        # normalize: at2 = psO[0:64] * (1/den) broadcast
                        for hp in range(2):
                            denb = den_p.tile([65, TOK], bf16, tag="den")
                            with nc.allow_low_precision(
                                    reason="softmax recip in bf16"):
                                nc.vector.reciprocal(denb[64:65, :],
                                                     psO[64:65, hp, :])
                            psb = br_ps.tile([64, TOK], f32, tag="br")
                            nc.tensor.matmul(psb[:], ones64[64:65, 0:64],
                                             denb[64:65, :],
                                             start=True, stop=True)
                            rb = rb_p.tile([64, TOK], bf16, tag="rb")
                            nc.vector.tensor_copy(rb[:], psb[:])
                            if hp == 0:
                                nc.vector.tensor_tensor(
                                    at2[0:64, hh, :],
                                    psO[0:64, 0, :], rb[:], ALU.mult)
                            else:
                                # DVE lanes can't shift partitions; bounce
                                # hp=1 rows to partitions 64.. via DMA
                                tmp = tmp_p.tile([64, TOK], bf16, tag="tmp")
                                nc.vector.tensor_tensor(
                                    tmp[:], psO[0:64, 1, :], rb[:], ALU.mult)
                                nc.sync.dma_start(at2[64:128, hh, :], tmp[:])

            # ---------- out projection + residual (in place on x_sb) ----------
            with tc.tile_pool(name="mm2_ps", bufs=2, space="PSUM") as mm2_ps:
                for no in range(2):
                    for tt in range(4):
                        ps = mm2_ps.tile([128, TOK], f32, tag="mm2")
                        for hh in range(8):
                            nc.tensor.matmul(
                                ps[:], at2[:, hh, 128 * tt:128 * (tt + 1)],
                                wout_sb[:, hh, no, :],
                                start=(hh == 0), stop=(hh == 7))
                        nc.vector.tensor_tensor(
                            x_sb[:, tt, TOK * no:TOK * (no + 1)], ps[:],
                            x_sb[:, tt, TOK * no:TOK * (no + 1)], ALU.add)

            # ---------- LN2 + transpose (reuses xlt) ----------
            with (
                tc.tile_pool(name="ln2", bufs=2) as ln2_pool,
                tc.tile_pool(name="tp2_ps", bufs=2, space="PSUM") as tp2_ps,
            ):
                layernorm_t(xlt, ln2_pool, tp2_ps)

            # ---------- MLP fc1 + gelu ----------
            with (
                tc.tile_pool(name="wfc1_p", bufs=2) as wfc1_p,
                tc.tile_pool(name="fc1_ps", bufs=2, space="PSUM") as fc1_ps,
            ):
                for ch in range(8):
                    wt = wfc1_p.tile([128, 4, 8, 128], bf16, tag="wfc1")
                    nc.sync.dma_start(wt[:], wfc1_d.ap()[:, 4 * ch:4 * (ch + 1),
                                                         :, :])
                    for fi in range(4):
                        f = 4 * ch + fi
                        ps = fc1_ps.tile([128, TOK], f32, tag="fc1")
                        for hc in range(8):
                            nc.tensor.matmul(ps[:], wt[:, fi, hc, :],
                                             xlt[:, hc, :],
                                             start=(hc == 0), stop=(hc == 7))
                        nc.scalar.activation(h1g[:, f, :], ps[:], AF.Gelu)

            # ---------- MLP fc2 + residual -> out ----------
            with (
                tc.tile_pool(name="wfc2_p", bufs=2) as wfc2_p,
                tc.tile_pool(name="fc2_ps", bufs=4, space="PSUM") as fc2_ps,
                tc.tile_pool(name="o_p", bufs=2) as o_p,
            ):
                out_view = out_d.ap().rearrange("(tt p) h -> p tt h", p=128)
                for no in range(2):
                    pss = [fc2_ps.tile([128, TOK], f32, tag="fc2",
                                       name=f"fc2ps_{no}_{tt}")
                           for tt in range(4)]
                    for ch in range(4):
                        wt = wfc2_p.tile([128, 8, TOK], bf16, tag="wfc2")
                        nc.sync.dma_start(
                            wt[:], wfc2_d.ap()[:, 8 * ch:8 * (ch + 1),
                                               TOK * no:TOK * (no + 1)])
                        for fi in range(8):
                            f = 8 * ch + fi
                            for tt in range(4):
                                nc.tensor.matmul(
                                    pss[tt],
                                    h1g[:, f, 128 * tt:128 * (tt + 1)],
                                    wt[:, fi, :],
                                    start=(f == 0), stop=(f == 31))
                    for tt in range(4):
                        o = o_p.tile([128, TOK], f32, tag="o")
                        nc.vector.tensor_tensor(
                            o[:], pss[tt],
                            x_sb[:, tt, TOK * no:TOK * (no + 1)], ALU.add)
                        nc.sync.dma_start(
                            out_view[:, tt, TOK * no:TOK * (no + 1)], o[:])

    nc.compile()
    return nc


def _host_prepare(x, ln1_w, ln2_w, w_qkv, w_out, w_fc1, w_fc2):
    """Fold LN weights into the following matmuls, cast to bf16, and
    rearrange weights into the layouts the kernel DMAs expect."""
    import ml_dtypes
    bf16 = ml_dtypes.bfloat16

    x = np.asarray(x, np.float32)
    wqkv_f = (np.asarray(ln1_w, np.float32)[:, None]
              * np.asarray(w_qkv, np.float32))
    wfc1_f = (np.asarray(ln2_w, np.float32)[:, None]
              * np.asarray(w_fc1, np.float32))

    # wqk[p, ot, hc, o]: ot 0-7 = q col blocks, 8-15 = k col blocks
    wqk = np.ascontiguousarray(
        wqkv_f[:, :2048].reshape(8, 128, 16, 128).transpose(1, 2, 0, 3)
    ).astype(bf16)
    # wv[p, hc, c] = wqkv_f[128*hc+p, 2048+c]
    wv = np.ascontiguousarray(
        wqkv_f[:, 2048:].reshape(8, 128, 1024).transpose(1, 0, 2)
    ).astype(bf16)
    # wout[64*hp+d, hh, no, o] = w_out[64*(2*hh+hp)+d, 512*no+o]
    wout = np.ascontiguousarray(
        np.asarray(w_out, np.float32).reshape(8, 2, 64, 2, TOK)
        .transpose(1, 2, 0, 3, 4).reshape(128, 8, 2, TOK)
    ).astype(bf16)
    # wfc1[p, f, hc, o] = wfc1_f[128*hc+p, 128*f+o]
    wfc1 = np.ascontiguousarray(
        wfc1_f.reshape(8, 128, 32, 128).transpose(1, 2, 0, 3)
    ).astype(bf16)
    # wfc2[p, f, c] = w_fc2[128*f+p, c]
    wfc2 = np.ascontiguousarray(
        np.asarray(w_fc2, np.float32).reshape(32, 128, 1024)
        .transpose(1, 0, 2)
    ).astype(bf16)

    # masks: mk[p, r, hp, qq] = 1 if 4p + r <= 4qq + k  (k = core rank)
    p_i = np.arange(128)[:, None, None, None]
    r_i = np.arange(4)[None, :, None, None]
    q_i = np.arange(128)[None, None, None, :]
    masks = [
        np.ascontiguousarray(np.broadcast_to(
            (4 * p_i + r_i <= 4 * q_i + k), (128, 4, 2, 128)).astype(bf16))
        for k in range(4)
    ]

    in_maps = []
    for i in range(NCORES):
        b, k = i // 4, i % 4
        in_maps.append({
            "xc": np.ascontiguousarray(x[b, k::4, :]),
            "wqk": wqk, "wv": wv, "wout": wout,
            "wfc1": wfc1, "wfc2": wfc2,
            "mk": masks[k],
        })
    return in_maps


def kernel(x, ln1_w, ln2_w, w_qkv, w_out, w_fc1, w_fc2):
    global _PROG
    from concourse.bass_utils import run_bass_kernel_spmd

    if _PROG is None:
        _PROG = _build_program()
    nc = _PROG

    in_maps = _host_prepare(x, ln1_w, ln2_w, w_qkv, w_out, w_fc1, w_fc2)
    res = run_bass_kernel_spmd(nc, in_maps, core_ids=list(range(NCORES)))
    out = np.empty((B, S, H), np.float32)
    for i in range(NCORES):
        b, k = i // 4, i % 4
        out[b, k::4, :] = res.results[i]["out"]
    return out
